# revision 1
# baseline (speedup 1.0000x reference)
"""Batched Householder reflection: s_new[b] = s[b] - 2*(v[b]@s[b])/(v[b]@v[b]) * v[b].

Full inputs v, s: [512, 512] f32. Sharded batch-parallel across 8 NeuronCores
(64 rows per core). Per core: rows on SBUF partitions, K=512 on the free axis.
v and s shards are stacked host-side into one [2, 64, 512] DRAM tensor.

Raw Bass (no Tile, no Block): this walrus codegen allows only ONE inline
sync-wait per instruction, so all cross-engine sync is standalone wait_ge.
The load is split across the two HWDGE engines (SP loads v, ACT loads s) so
the two 128KB transfers overlap; the store is likewise split K-wise across
SP/ACT. ACT prewarms its Square activation table while the DMAs fly.
  dot = rowsum(v*s)   (DVE scalar_tensor_tensor with accum_out)
  nsq = rowsum(v*v)   (ACT Square activation with accum_out, runs in parallel)
  coef = -2*dot/nsq   (tiny per-partition ops)
  out = coef*v + s    (one fused DVE op, per-partition scalar broadcast)
"""

import numpy as np

B, K = 512, 512
N_CORES = 8
B_LOC = B // N_CORES  # 64 rows per core

_nc = None


def _build():
    import concourse.bass as bass
    from concourse import mybir

    nc = bass.Bass("TRN2", debug=False, num_devices=N_CORES, num_swdge_queues=2)
    f32 = mybir.dt.float32

    vs = nc.dram_tensor("vs", [2, B_LOC, K], f32, kind="ExternalInput").ap()
    out = nc.dram_tensor("out", [B_LOC, K], f32, kind="ExternalOutput").ap()

    vs_t = nc.alloc_sbuf_tensor("vs_t", [B_LOC, 2, K], f32).ap()
    o_t = nc.alloc_sbuf_tensor("o_t", [B_LOC, K], f32).ap()
    junk_vs = nc.alloc_sbuf_tensor("junk_vs", [B_LOC, K], f32).ap()
    junk_vv = nc.alloc_sbuf_tensor("junk_vv", [B_LOC, K], f32).ap()
    warm = nc.alloc_sbuf_tensor("warm", [B_LOC, 1], f32).ap()
    dot = nc.alloc_sbuf_tensor("dot", [B_LOC, 1], f32).ap()
    nsq = nc.alloc_sbuf_tensor("nsq", [B_LOC, 1], f32).ap()
    rcp = nc.alloc_sbuf_tensor("rcp", [B_LOC, 1], f32).ap()
    coef = nc.alloc_sbuf_tensor("coef", [B_LOC, 1], f32).ap()

    dma_in = nc.alloc_semaphore("dma_in")
    act_done = nc.alloc_semaphore("act_done")
    dve_sem = nc.alloc_semaphore("dve_sem")
    dve_done = nc.alloc_semaphore("dve_done")
    dma_out = nc.alloc_semaphore("dma_out")

    mult = mybir.AluOpType.mult
    add = mybir.AluOpType.add
    Square = mybir.ActivationFunctionType.Square

    sp, act, ve = nc.sync, nc.scalar, nc.vector
    v_t = vs_t[:, 0, :]
    s_t = vs_t[:, 1, :]
    zero64 = nc.const_aps.scalar_like(0.0, dot[:])

    # ---- loads ----
    # Each issue engine's dynamic DMA queue serializes its transfers at
    # ~28 GB/s, and each dma_start costs ~600ns of issue time on the engine.
    # So fan the 256KB input across FOUR streams: SP and ACT (one HWDGE queue
    # each) take the top row-halves, Pool (SWDGE) takes the bottom halves.
    pl = nc.gpsimd
    HB = B_LOC // 2  # 32 rows
    sp.dma_start(out=vs_t[:HB, 0, :], in_=vs[0, :HB, :]).then_inc(dma_in, 16)
    act.dma_start(out=vs_t[:HB, 1, :], in_=vs[1, :HB, :]).then_inc(dma_in, 16)
    pl.dma_start(out=vs_t[HB:, 0, :], in_=vs[0, HB:, :]).then_inc(dma_in, 16)
    pl.dma_start(out=vs_t[HB:, 1, :], in_=vs[1, HB:, :]).then_inc(dma_in, 16)

    # ACT: prewarm the Square table while the DMAs are in flight
    act.activation(out=warm[:], in_=zero64, func=Square)
    act.wait_ge(dma_in, 64)
    act.activation(out=junk_vv[:], in_=v_t, func=Square, accum_out=nsq[:]).then_inc(
        act_done, 1
    )

    # DVE chain
    ve.wait_ge(dma_in, 64)
    ve.scalar_tensor_tensor(
        out=junk_vs[:],
        in0=v_t,
        scalar=1.0,
        in1=s_t,
        op0=mult,
        op1=mult,
        accum_out=dot[:],
    ).then_inc(dve_sem, 1)
    ve.wait_ge(act_done, 1)
    ve.reciprocal(out=rcp[:], in_=nsq[:]).then_inc(dve_sem, 1)
    # DVE writes are not visible to the next DVE instruction without a
    # completion wait (in-order issue != in-order write visibility).
    ve.wait_ge(dve_sem, 2)
    ve.scalar_tensor_tensor(
        out=coef[:], in0=dot[:], scalar=-2.0, in1=rcp[:], op0=mult, op1=mult
    ).then_inc(dve_sem, 1)
    ve.wait_ge(dve_sem, 3)
    ve.scalar_tensor_tensor(
        out=o_t[:],
        in0=v_t,
        scalar=coef[:],
        in1=s_t,
        op0=mult,
        op1=add,
    ).then_inc(dve_done, 2)

    # ---- stores: three streams (SP / ACT / Pool) ----
    # ACT's store issue is measurably slower (activation-pipe drain before
    # descriptor gen), so it gets the smallest chunk.
    sp.wait_ge(dve_done, 2)
    sp.dma_start(out=out[0:24, :], in_=o_t[0:24, :]).then_inc(dma_out, 16)
    act.wait_ge(dve_done, 2)
    act.dma_start(out=out[24:42, :], in_=o_t[24:42, :]).then_inc(dma_out, 16)
    pl.wait_ge(dve_done, 2)
    pl.dma_start(out=out[42:64, :], in_=o_t[42:64, :]).then_inc(dma_out, 16)

    # SP resets semaphores for re-execution (PJRT reuses the loaded NEFF;
    # semaphores persist between executions). Sems whose waiters have
    # provably passed (everything up to dve_done) clear while the store
    # transfers drain; dve_done/dma_out clear after the final wait proves
    # Pool and ACT passed their dve_done waits too.
    sp.wait_ge(dve_done, 2)
    for sem in (dma_in, act_done, dve_sem):
        sp.sem_clear(sem)
    sp.wait_ge(dma_out, 48)
    sp.sem_clear(dve_done)
    sp.sem_clear(dma_out)

    return nc


def kernel(i=None, v=None, s=None, **_):
    global _nc
    from concourse.bass_utils import run_bass_kernel_spmd

    if _nc is None:
        _nc = _build()

    v = np.asarray(v, dtype=np.float32)
    s = np.asarray(s, dtype=np.float32)
    in_maps = [
        {
            "vs": np.ascontiguousarray(
                np.stack(
                    [v[c * B_LOC : (c + 1) * B_LOC], s[c * B_LOC : (c + 1) * B_LOC]]
                )
            )
        }
        for c in range(N_CORES)
    ]
    res = run_bass_kernel_spmd(_nc, in_maps, core_ids=list(range(N_CORES)))
    return np.concatenate([r["out"] for r in res.results], axis=0)



# revision 3
# speedup vs baseline: 1.0711x; 1.0711x over previous
"""Batched Householder reflection: s_new[b] = s[b] - 2*(v[b]@s[b])/(v[b]@v[b]) * v[b].

Full inputs v, s: [512, 512] f32. Sharded batch-parallel across 8 NeuronCores
(64 rows per core). Per core the K=512 axis is split in half and interleaved
inside 32-partition quadrants so every DVE/DMA op runs at full 128-partition
width AND the cross-partition combine is expressible with STREAM_SHUFFLE
(which permutes only within 32-partition quadrants):
    partition 32q + 16h + j  <-  row (16q + j), K-half h      (q<4, h<2, j<16)

Engines: SP+ACT issue HWDGE DMAs, DVE does all compute. No gpsimd DMA (SWDGE
is slow), no ACT activations (avoids the ~1.3us ACT_TABLE_LOAD).

DVE chain (TRN2 walrus requires equal base partitions for all SBUF operands
of tensor ops, and has no float divide -- hence shuffle + reciprocal):
  a:    nsq partials  acc[:,1] = rowsum(v*v)        (starts once v lands)
  b:    dot partials  acc[:,0] = rowsum(-2*v*s)     (once s lands)
  shuf: accs = quadrant-half-swap(acc)               (one op, both columns)
  c2n:  nsqf = acc[:,1] + accs[:,1]                  (stt scalar-AP fusion)
  rcp:  rn = 1/nsqf
  coef: coef = (acc[:,0] + accs[:,0]) * rn           (stt scalar-AP fusion)
  e:    ot = coef*v + s                              (single 128-part op)
"""

import numpy as np

B, K = 512, 512
N_CORES = 8
B_LOC = B // N_CORES  # 64 rows per core
KH = K // 2  # 256

_nc = None


def _build():
    import concourse.bass as bass
    from concourse import mybir

    nc = bass.Bass("TRN2", debug=False, num_devices=N_CORES)
    f32 = mybir.dt.float32

    vs = nc.dram_tensor("vs", [2, 128, KH], f32, kind="ExternalInput").ap()
    out = nc.dram_tensor("out", [128, KH], f32, kind="ExternalOutput").ap()

    vst = nc.alloc_sbuf_tensor("vst", [128, 2, KH], f32).ap()
    ot = nc.alloc_sbuf_tensor("ot", [128, KH], f32).ap()
    junk0 = nc.alloc_sbuf_tensor("junk0", [128, KH], f32).ap()
    junk1 = nc.alloc_sbuf_tensor("junk1", [128, KH], f32).ap()
    acc = nc.alloc_sbuf_tensor("acc", [128, 2], f32).ap()
    accs = nc.alloc_sbuf_tensor("accs", [128, 2], f32).ap()
    nsqf = nc.alloc_sbuf_tensor("nsqf", [128, 1], f32).ap()
    rn = nc.alloc_sbuf_tensor("rn", [128, 1], f32).ap()
    coef = nc.alloc_sbuf_tensor("coef", [128, 1], f32).ap()

    dma_in = nc.alloc_semaphore("dma_in")
    dve_sem = nc.alloc_semaphore("dve_sem")
    dve_done = nc.alloc_semaphore("dve_done")
    dma_out = nc.alloc_semaphore("dma_out")

    mult = mybir.AluOpType.mult
    add = mybir.AluOpType.add

    sp, act, ve = nc.sync, nc.scalar, nc.vector
    v_t = vst[:, 0, :]
    s_t = vst[:, 1, :]
    ones = nc.const_aps.aps[(f32, 1.0)]

    # ---- loads: v then s on SP's FIFO HWDGE queue (v lands ~300ns early,
    # hiding the nsq-partials op in the s-transfer tail) ----
    sp.dma_start(out=v_t, in_=vs[0]).then_inc(dma_in, 16)
    sp.dma_start(out=s_t, in_=vs[1]).then_inc(dma_in, 16)

    # ---- DVE chain ----
    ve.wait_ge(dma_in, 16)
    ve.scalar_tensor_tensor(  # a: nsq partials
        out=junk0[:], in0=v_t, scalar=1.0, in1=v_t,
        op0=mult, op1=mult, accum_out=acc[:, 1:2],
    ).then_inc(dve_sem, 1)
    ve.wait_ge(dma_in, 32)
    ve.scalar_tensor_tensor(  # b: -2*dot partials
        out=junk1[:], in0=v_t, scalar=-2.0, in1=s_t,
        op0=mult, op1=mult, accum_out=acc[:, 0:1],
    ).then_inc(dve_sem, 1)
    ve.wait_ge(dve_sem, 2)
    # swap quadrant halves (partition 32q+16h+j <-> 32q+16(1-h)+j)
    ve.stream_shuffle(
        out=accs[:], in_=acc[:], mask=list(range(16, 32)) + list(range(0, 16))
    ).then_inc(dve_sem, 1)
    ve.wait_ge(dve_sem, 3)
    ve.scalar_tensor_tensor(  # c2n: nsqf = nsq_lo + nsq_hi
        out=nsqf[:], in0=acc[:, 1:2], scalar=accs[:, 1:2], in1=ones,
        op0=add, op1=mult,
    ).then_inc(dve_sem, 1)
    ve.wait_ge(dve_sem, 4)
    ve.reciprocal(out=rn[:], in_=nsqf[:]).then_inc(dve_sem, 1)
    ve.wait_ge(dve_sem, 5)
    ve.scalar_tensor_tensor(  # coef = (-2*dot) * (1/nsq)
        out=coef[:], in0=acc[:, 0:1], scalar=accs[:, 0:1], in1=rn[:],
        op0=add, op1=mult,
    ).then_inc(dve_sem, 1)
    ve.wait_ge(dve_sem, 6)
    ve.scalar_tensor_tensor(  # e: out = coef*v + s
        out=ot[:], in0=v_t, scalar=coef[:], in1=s_t, op0=mult, op1=add
    ).then_inc(dve_done, 1)

    # ---- stores: SP low half (even SDMA engines) / ACT high half (odd) ----
    sp.wait_ge(dve_done, 1)
    sp.dma_start(out=out[0:64, :], in_=ot[0:64, :]).then_inc(dma_out, 16)
    act.wait_ge(dve_done, 1)
    act.dma_start(out=out[64:128, :], in_=ot[64:128, :]).then_inc(dma_out, 16)

    # ---- semaphore reset for NEFF re-execution ----
    sp.sem_clear(dma_in)  # DVE passed both dma_in waits (dve_done fired)
    sp.sem_clear(dve_sem)
    sp.wait_ge(dma_out, 32)  # both stores landed (so ACT passed its wait too)
    sp.sem_clear(dve_done)
    sp.sem_clear(dma_out)

    return nc


def _interleave(x: np.ndarray) -> np.ndarray:
    """[64,512] -> [128,256] quadrant-interleaved K-split."""
    return np.ascontiguousarray(
        x.reshape(4, 16, 2, KH).transpose(0, 2, 1, 3).reshape(128, KH)
    )


def _deinterleave(x: np.ndarray) -> np.ndarray:
    """[128,256] quadrant-interleaved -> [64,512]."""
    return x.reshape(4, 2, 16, KH).transpose(0, 2, 1, 3).reshape(B_LOC, K)


def make_in_maps(v: np.ndarray, s: np.ndarray) -> list[dict]:
    v = np.asarray(v, dtype=np.float32)
    s = np.asarray(s, dtype=np.float32)
    return [
        {
            "vs": np.ascontiguousarray(
                np.stack(
                    [
                        _interleave(v[c * B_LOC : (c + 1) * B_LOC]),
                        _interleave(s[c * B_LOC : (c + 1) * B_LOC]),
                    ]
                )
            )
        }
        for c in range(N_CORES)
    ]


def unpack_out(res_list) -> np.ndarray:
    return np.ascontiguousarray(
        np.concatenate([_deinterleave(r["out"]) for r in res_list], axis=0)
    )


def kernel(i=None, v=None, s=None, **_):
    global _nc
    from concourse.bass_utils import run_bass_kernel_spmd

    if _nc is None:
        _nc = _build()

    res = run_bass_kernel_spmd(_nc, make_in_maps(v, s), core_ids=list(range(N_CORES)))
    return unpack_out(res.results)
